# revision 5
# baseline (speedup 1.0000x reference)
"""Trainium2 Bass kernel for nn_CFDFVnewGCN (6-layer FVnewConv GNN).

Strategy: shard destination nodes (and their incoming edges) across 8 cores.
Nodes are permuted/degree-balanced into 49 windows of <=128 nodes per core.
X rows are laid out in two chunks (chunk0 = windows [0,W0), chunk1 = rest) so
each layer's AllGather is split in two and gathers depend only on the chunk
they read (chunk-relative int16 indices). All matmul/elementwise data is bf16
(PSUM accumulation f32): per 128-edge tile the scaling matmul (edge_attr
stationary, bias folded as 7th K-row) feeds an ACT relu (+DVE tail), a DVE
message multiply, and a one-hot scatter matmul accumulating aggr[window, D]
in PSUM. Per window: PE transposes of aggr + fused output matmul (bias via
const ones row), relu/tanh, DMA out.
"""
import sys
import numpy as np

for _p in ('/opt/trn_rl_repo', '/root/.axon_site/_ro/trn_rl_repo'):
    if _p not in sys.path:
        sys.path.insert(0, _p)

import concourse.bacc as bacc
import concourse.mybir as mybir
import concourse.tile as tile
from concourse.bass_utils import run_bass_kernel_spmd

import ml_dtypes

BF16NP = ml_dtypes.bfloat16
F32 = mybir.dt.float32
BF16 = mybir.dt.bfloat16
I16 = mybir.dt.int16
I32 = mybir.dt.int32
COPY = mybir.ActivationFunctionType.Copy
RELU = mybir.ActivationFunctionType.Relu
TANH = mybir.ActivationFunctionType.Tanh
MULT = mybir.AluOpType.mult
MAX = mybir.AluOpType.max
ISEQ = mybir.AluOpType.is_equal

NCORES = 8


class Cfg:
    def __init__(self, n_nodes=50000, n_edges=200000, hid=512, hs=3, ea=6,
                 out=3, w0=25):
        self.N = n_nodes
        self.E = n_edges
        self.HID = hid
        self.HS = hs
        self.EA = ea
        self.OUT = out
        self.NPC = self.N // NCORES              # nodes per core
        self.NWIN = (self.NPC + 127) // 128      # windows per core
        self.WSIZES = [128] * (self.NWIN - 1) + [self.NPC - 128 * (self.NWIN - 1)]
        # two src chunks: chunk0 = windows [0, W0), chunk1 = rest
        self.W0 = w0
        self.CWIN = [(0, w0), (w0, self.NWIN)]
        self.CROWS = [sum(self.WSIZES[a:b]) for a, b in self.CWIN]
        self.CBASE = [0, NCORES * self.CROWS[0]]
        assert NCORES * self.CROWS[0] < 32768
        assert NCORES * self.CROWS[1] < 32768
        # layer table
        self.LAYERS = []
        for name in ['p0', 'p1', 'p2', 'c0', 'c1', 'c2']:
            if name == 'p0':
                ic, g, oc = 7, 7, hid
            elif name == 'c0':
                ic, g, oc = hid + 4, hid, hid
            elif name == 'c2':
                ic, g, oc = hid + 1, hid, out
            else:
                ic, g, oc = hid + 1, hid, hid
            D = ic * hs
            DP = D + (D % 2)
            OCP = oc + (oc % 2)
            if oc == out:
                OCP = 4
            self.LAYERS.append(dict(name=name, ic=ic, g=g, oc=oc, D=D, DP=DP,
                                    OCP=OCP, relu=(name != 'c2')))


def _col2orig(cfg, lay):
    """Map plane-major padded column -> original scaling index j=i*HS+h, -1=pad."""
    HS, g, ic, DP = cfg.HS, lay['g'], lay['ic'], lay['DP']
    m = np.full(DP, -1, np.int64)
    if lay['name'] == 'p0':
        for h in range(HS):
            for i in range(g):
                m[h * g + i] = i * HS + h
    elif lay['name'] == 'c0':
        for h in range(HS):
            for i in range(g):
                m[h * g + i] = (3 + i) * HS + h          # fine_x at xc dims 3..
        for f in range(3):
            for h in range(HS):
                m[HS * g + 3 * f + h] = f * HS + h        # fyo
        for h in range(HS):
            m[HS * g + 9 + h] = (ic - 1) * HS + h         # na
    else:
        for h in range(HS):
            for i in range(g):
                m[h * g + i] = i * HS + h
        for h in range(HS):
            m[HS * g + h] = g * HS + h                    # na
    return m


def _balance(items_deg, caps):
    """Greedy: assign items (sorted by degree desc) to bins with capacity,
    minimizing max degree sum. Returns bin index per item."""
    order = np.argsort(-items_deg, kind='stable')
    nbins = len(caps)
    load = np.zeros(nbins)
    cnt = np.zeros(nbins, np.int64)
    out = np.zeros(len(items_deg), np.int64)
    import heapq
    heap = [(0.0, b) for b in range(nbins)]
    heapq.heapify(heap)
    for it in order:
        while True:
            l, b = heapq.heappop(heap)
            if cnt[b] < caps[b]:
                break
        out[it] = b
        cnt[b] += 1
        load[b] += items_deg[it]
        if cnt[b] < caps[b]:
            heapq.heappush(heap, (load[b], b))
    return out


def _preprocess(cfg, inputs):
    N, E, HS = cfg.N, cfg.E, cfg.HS
    ei = np.asarray(inputs['edge_index'])
    src = ei[0].astype(np.int64)
    dst = ei[1].astype(np.int64)
    deg = np.bincount(dst, minlength=N).astype(np.float64)

    node_core = _balance(deg, [cfg.NPC] * NCORES)
    node_win = np.zeros(N, np.int64)
    node_slot = np.zeros(N, np.int64)
    for c in range(NCORES):
        nodes = np.where(node_core == c)[0]
        w = _balance(deg[nodes], cfg.WSIZES)
        node_win[nodes] = w
        for wi in range(cfg.NWIN):
            sel = nodes[w == wi]
            node_slot[sel] = np.arange(len(sel))

    # x row layout: chunk-major, core-major within chunk, window-major in core
    node_row = node_win * 128 + node_slot           # within-core row (out rows)
    wchunk = np.where(np.arange(cfg.NWIN) < cfg.W0, 0, 1)
    wofs = np.zeros(cfg.NWIN, np.int64)
    for k, (a, b) in enumerate(cfg.CWIN):
        for w in range(a, b):
            wofs[w] = sum(cfg.WSIZES[a:w])
    k_of = wchunk[node_win]
    xrow = (np.array(cfg.CBASE)[k_of] + node_core * np.array(cfg.CROWS)[k_of]
            + wofs[node_win] + node_slot)
    xrow_src = xrow[src]
    src_chunk = (xrow_src >= cfg.CBASE[1]).astype(np.int64)

    # edge buckets per (core, window, src-chunk)
    ec = node_core[dst]
    ew = node_win[dst]

    kA = np.zeros(cfg.NWIN, np.int64)
    kB = np.zeros(cfg.NWIN, np.int64)
    cntA = np.zeros((NCORES, cfg.NWIN), np.int64)
    cntB = np.zeros((NCORES, cfg.NWIN), np.int64)
    np.add.at(cntA, (ec[src_chunk == 0], ew[src_chunk == 0]), 1)
    np.add.at(cntB, (ec[src_chunk == 1], ew[src_chunk == 1]), 1)
    for w in range(cfg.NWIN):
        kA[w] = int(np.ceil(cntA[:, w].max() / 128))
        kB[w] = int(np.ceil(cntB[:, w].max() / 128))
    tw = kA + kB
    tbase = np.concatenate([[0], np.cumsum(tw)])
    T = int(tbase[-1])

    ea_np = np.asarray(inputs['edge_attr'], np.float32)
    na_np = np.asarray(inputs['node_attr'], np.float32).reshape(-1)
    fyo_np = np.asarray(inputs['fine_y_orig'], np.float32)

    per_core = []
    for c in range(NCORES):
        ea_s = np.zeros((7, T * 128), np.float32)
        idx_s = np.zeros((16, T * 8), np.int16)
        dst_s = np.full((128, T), 999.0, np.float32)
        na_s = np.zeros((128, T), np.float32)
        fyo_s = np.zeros((128, 3 * T), np.float32)
        for w in range(cfg.NWIN):
            t0 = int(tbase[w])
            for side, ktiles, toff in ((0, int(kA[w]), t0), (1, int(kB[w]), t0 + int(kA[w]))):
                nslots = ktiles * 128
                if nslots == 0:
                    continue
                edges = np.where((ec == c) & (ew == w) & (src_chunk == side))[0]
                assert len(edges) <= nslots
                iv = np.zeros(nslots, np.int64)
                iv[:len(edges)] = xrow_src[edges] - cfg.CBASE[side]
                assert iv.min() >= 0 and iv.max() < 32768
                jj = np.arange(nslots)
                idx_s[jj % 16, toff * 8 + jj // 16] = iv.astype(np.int16)
                if len(edges):
                    e_jj = jj[:len(edges)]
                    e_tt = toff + e_jj // 128
                    e_pp = e_jj % 128
                    ea_s[0:6, e_tt * 128 + e_pp] = ea_np[edges].T
                    ea_s[6, e_tt * 128 + e_pp] = 1.0
                    dst_s[e_pp, e_tt] = node_slot[dst[edges]]
                    na_s[e_pp, e_tt] = na_np[src[edges]]
                    fyo_s[e_pp.repeat(3), (e_tt * 3).repeat(3)
                          + np.tile([0, 1, 2], len(edges))] = fyo_np[src[edges]].ravel()
        per_core.append(dict(ea_s=ea_s.astype(BF16NP), idx_s=np.tile(idx_s, (8, 1)),
                             dst_s=dst_s, na_s=na_s, fyo_s=fyo_s))

    # xc0 buffer: [N, 64] f32 in x-row order: cols [x(5), sdf, na, 0...]
    # (gather elem_size must be a multiple of 256 bytes -> 64 f32 cols)
    x_np = np.asarray(inputs['x'], np.float32)
    sdf_np = np.asarray(inputs['sdf'], np.float32)
    xc0 = np.zeros((N, 64), np.float32)
    xc0[xrow, 0:5] = x_np
    xc0[xrow, 5] = sdf_np[:, 0]
    xc0[xrow, 6] = na_np

    wts = {}
    for lay in cfg.LAYERS:
        nm = lay['name']
        win = np.asarray(inputs[f'win_{nm}'], np.float32)
        bin_ = np.asarray(inputs[f'bin_{nm}'], np.float32)
        wout = np.asarray(inputs[f'wout_{nm}'], np.float32)
        bout = np.asarray(inputs[f'bout_{nm}'], np.float32)
        m = _col2orig(cfg, lay)
        DP, OCP = lay['DP'], lay['OCP']
        winT = np.zeros((7, DP), np.float32)
        sel = m >= 0
        winT[0:cfg.EA, sel] = win[m[sel]].T
        winT[6, sel] = bin_[m[sel]]
        woutT = np.zeros((DP + 1, OCP), np.float32)
        woutT[np.where(sel)[0], 0:lay['oc']] = wout[:, m[sel]].T
        woutT[DP, 0:lay['oc']] = bout
        wts[f'winT_{nm}'] = winT.astype(BF16NP)
        wts[f'woutT_{nm}'] = woutT.astype(BF16NP)

    struct = dict(kA=kA, kB=kB, tw=tw, tbase=tbase, T=T, TWMAX=int(tw.max()))
    asm = dict(node_core=node_core, node_row=node_row)
    return struct, per_core, wts, xc0, asm


def _build(cfg, struct, repeat=1):
    kA, kB, tw, tbase, T = (struct['kA'], struct['kB'], struct['tw'],
                            struct['tbase'], struct['T'])
    TWMAX = struct['TWMAX']
    HID = cfg.HID
    R0 = cfg.CBASE[1]

    nc = bacc.Bacc("TRN2", target_bir_lowering=False, debug=False,
                   enable_asserts=True, num_devices=NCORES,
                   num_swdge_queues=4)
    ea_in = nc.dram_tensor("ea_s", [7, T * 128], BF16, kind="ExternalInput").ap()
    idx_in = nc.dram_tensor("idx_s", [128, T * 8], I16, kind="ExternalInput").ap()
    dst_in = nc.dram_tensor("dst_s", [128, T], F32, kind="ExternalInput").ap()
    na_in = nc.dram_tensor("na_s", [128, T], F32, kind="ExternalInput").ap()
    fyo_in = nc.dram_tensor("fyo_s", [128, 3 * T], F32, kind="ExternalInput").ap()
    xc0_in = nc.dram_tensor("xc0_in", [cfg.N, 64], F32, kind="ExternalInput").ap()
    win_ins = {}
    wout_ins = {}
    for lay in cfg.LAYERS:
        nm = lay['name']
        win_ins[nm] = nc.dram_tensor(f"winT_{nm}", [7, lay['DP']], BF16,
                                     kind="ExternalInput").ap()
        wout_ins[nm] = nc.dram_tensor(f"woutT_{nm}", [lay['DP'] + 1, lay['OCP']],
                                      BF16, kind="ExternalInput").ap()
    out_fin = nc.dram_tensor("out_final", [cfg.NPC, cfg.OUT], F32,
                             kind="ExternalOutput").ap()

    DPMAX = max(l['DP'] for l in cfg.LAYERS)
    NFULLMAX = max(l['DP'] // 128 for l in cfg.LAYERS)
    TAILMAX = max(l['DP'] - 128 * (l['DP'] // 128) for l in cfg.LAYERS)
    OCPMAX = max(l['OCP'] for l in cfg.LAYERS)

    with tile.TileContext(nc) as tc:
        with (
            tc.tile_pool(name="cst", bufs=1) as cst,
            tc.tile_pool(name="sbw", bufs=2) as sbw,
            tc.tile_pool(name="gst", bufs=2) as gst,
            tc.tile_pool(name="eap", bufs=2) as eap,
            tc.tile_pool(name="scp", bufs=2) as scp,
            tc.tile_pool(name="msgp", bufs=2) as msgp,
            tc.tile_pool(name="Sp", bufs=2) as Sp,
            tc.tile_pool(name="agsp", bufs=2) as agsp,
            tc.tile_pool(name="agtp", bufs=2) as agtp,
            tc.tile_pool(name="outp", bufs=2) as outp,
            tc.tile_pool(name="ps_sc", bufs=2, space="PSUM") as ps_sc,
            tc.tile_pool(name="ps_ag", bufs=1, space="PSUM") as ps_ag,
            tc.tile_pool(name="ps_tp", bufs=1, space="PSUM") as ps_tp,
            tc.tile_pool(name="ps_om", bufs=1, space="PSUM") as ps_om,
            tc.tile_pool(name="dram", bufs=1, space="DRAM") as dram,
        ):
            # ---- constants
            iota_i = cst.tile([128, 128], I32)
            nc.gpsimd.iota(iota_i[:, :], pattern=[[1, 128]], base=0,
                           channel_multiplier=0)
            iota_bf = cst.tile([128, 128], BF16)
            nc.vector.tensor_copy(iota_bf[:, :], iota_i[:, :])
            iota_p = cst.tile([128, 1], I32)
            nc.gpsimd.iota(iota_p[:, :], pattern=[[1, 1]], base=0,
                           channel_multiplier=1)
            iota_pf = cst.tile([128, 1], F32)
            nc.vector.tensor_copy(iota_pf[:, :], iota_p[:, :])
            ident = cst.tile([128, 128], BF16)
            nc.vector.tensor_scalar(out=ident[:, :], in0=iota_bf[:, :],
                                    scalar1=iota_pf[:, :], scalar2=None,
                                    op0=ISEQ)
            ones_i = cst.tile([1, 128], I32)
            nc.gpsimd.iota(ones_i[:, :], pattern=[[0, 128]], base=1,
                           channel_multiplier=0)
            ones = cst.tile([1, 128], BF16)
            nc.vector.tensor_copy(ones[:, :], ones_i[:, :])

            # ---- static per-slot data (resident)
            dst_sb = cst.tile([128, T], F32)
            nc.sync.dma_start(out=dst_sb[:, :], in_=dst_in[:, :])
            na_sb = cst.tile([128, T], F32)
            nc.sync.dma_start(out=na_sb[:, :], in_=na_in[:, :])
            fyo_sb = cst.tile([128, 3 * T], F32)
            nc.sync.dma_start(out=fyo_sb[:, :], in_=fyo_in[:, :])
            idx_sb = cst.tile([128, T * 8], I16)
            nc.sync.dma_start(out=idx_sb[:, :], in_=idx_in[:, :])

            # ---- DRAM buffers
            xc0b = dram.tile([cfg.N, 64], F32)
            nc.sync.dma_start(out=xc0b[:, :], in_=xc0_in[:, :])
            xstate = {}

            def emit_layer(lay, rep):
                nm, DP, OCP, g = lay['name'], lay['DP'], lay['OCP'], lay['g']
                nfull = DP // 128
                tailk = DP - nfull * 128
                pieces = [(i, min(i + 512, DP)) for i in range(0, DP, 512)]
                if nm == 'p0':
                    gsrcA, gsrcB, gcols = xc0b[0:R0, :], xc0b[R0:cfg.N, :], 64
                else:
                    gsrcA, gsrcB = xstate['cur']
                    gsrcA, gsrcB, gcols = gsrcA[:, :], gsrcB[:, :], HID

                # layer weights (bf16 direct)
                winT = sbw.tile([7, DPMAX], BF16, tag="winT")
                nc.sync.dma_start(out=winT[:, 0:DP], in_=win_ins[nm][:, :])
                wt = sbw.tile([128, NFULLMAX * OCPMAX], BF16, tag="wt")
                for ci in range(nfull):
                    nc.sync.dma_start(
                        out=wt[:, ci * OCP:ci * OCP + OCP],
                        in_=wout_ins[nm][ci * 128:(ci + 1) * 128, :])
                wtail = sbw.tile([TAILMAX, OCPMAX], BF16, tag="wtail")
                if tailk:
                    nc.sync.dma_start(
                        out=wtail[0:tailk, 0:OCP],
                        in_=wout_ins[nm][nfull * 128:nfull * 128 + tailk, :])
                wbias = sbw.tile([1, OCPMAX], BF16, tag="wbias")
                nc.sync.dma_start(out=wbias[:, 0:OCP],
                                  in_=wout_ins[nm][DP:DP + 1, :])

                if nm != 'c2':
                    Xout0 = dram.tile([R0, HID], BF16, addr_space="Shared",
                                      tag=f"Xa_{nm}", name=f"Xa_{nm}_{rep}")
                    Xout1 = dram.tile([cfg.N - R0, HID], BF16,
                                      addr_space="Shared",
                                      tag=f"Xb_{nm}", name=f"Xb_{nm}_{rep}")
                    xstate['cur'] = (Xout0, Xout1)
                    oslices = []
                    for k, r in enumerate(cfg.CROWS):
                        t_ = dram.tile([r, HID], BF16, tag=f"osl_{nm}_{k}",
                                       name=f"osl_{nm}_{k}_{rep}")
                        oslices.append(t_)

                for w in range(cfg.NWIN):
                    nt = int(tw[w])
                    t0 = int(tbase[w])
                    ka, kb = int(kA[w]), int(kB[w])
                    wsz = cfg.WSIZES[w]
                    wchunk = 0 if w < cfg.W0 else 1
                    # gathers: separate A/B staging tiles for independent deps
                    xdt_l = F32 if nm == 'p0' else BF16
                    xstA = gst.tile([128, TWMAX, gcols], xdt_l,
                                    tag=f"xstA_{nm == 'p0'}")
                    xstB = gst.tile([128, TWMAX, gcols], xdt_l,
                                    tag=f"xstB_{nm == 'p0'}")
                    if ka:
                        nc.gpsimd.dma_gather(
                            out_ap=xstA[:, 0:ka, 0:gcols],
                            in_ap=gsrcA,
                            idxs_ap=idx_sb[:, t0 * 8:(t0 + ka) * 8],
                            num_idxs=ka * 128, num_idxs_reg=ka * 128,
                            elem_size=gcols, queue_num=0)
                    if kb:
                        nc.gpsimd.dma_gather(
                            out_ap=xstB[:, 0:kb, 0:gcols],
                            in_ap=gsrcB,
                            idxs_ap=idx_sb[:, (t0 + ka) * 8:(t0 + nt) * 8],
                            num_idxs=kb * 128, num_idxs_reg=kb * 128,
                            elem_size=gcols, queue_num=0)
                    # edge attr (bf16, direct matmul operand)
                    ear = eap.tile([7, TWMAX * 128], BF16, tag="ear")
                    nc.sync.dma_start(out=ear[:, 0:nt * 128],
                                      in_=ea_in[:, t0 * 128:(t0 + nt) * 128])

                    agps = ps_ag.tile([128, DPMAX], F32, tag="agps")
                    for t in range(nt):
                        tg = t0 + t
                        xst = xstA[:, t, :] if t < ka else xstB[:, t - ka, :]
                        sc = scp.tile([128, DPMAX], BF16, tag="sc")
                        scps_l = []
                        for (p0_, p1_) in pieces:
                            scps = ps_sc.tile([128, 512], F32, tag="scps")
                            nc.tensor.matmul(
                                scps[:, 0:p1_ - p0_],
                                ear[:, t * 128:(t + 1) * 128],
                                winT[:, p0_:p1_], start=True, stop=True)
                            if p1_ - p0_ > 64:
                                nc.scalar.activation(sc[:, p0_:p1_],
                                                     scps[:, 0:p1_ - p0_], RELU)
                            else:
                                # tiny tail piece: relu on DVE
                                nc.vector.tensor_scalar(
                                    out=sc[:, p0_:p1_],
                                    in0=scps[:, 0:p1_ - p0_],
                                    scalar1=0.0, scalar2=None, op0=MAX)
                        msg = msgp.tile([128, DPMAX], BF16, tag="msg")
                        if nm == 'p0':
                            for h in range(cfg.HS):
                                lo = h * g
                                hi = lo + g + (1 if h == cfg.HS - 1 else 0)
                                nc.vector.tensor_tensor(
                                    out=msg[:, lo:hi], in0=sc[:, lo:hi],
                                    in1=xst[:, 0:hi - lo], op=MULT)
                        else:
                            for h in range(cfg.HS):
                                nc.vector.tensor_tensor(
                                    out=msg[:, h * g:(h + 1) * g],
                                    in0=sc[:, h * g:(h + 1) * g],
                                    in1=xst[:, 0:g], op=MULT)
                            nab = cfg.HS * g + (9 if nm == 'c0' else 0)
                            nc.vector.tensor_tensor(
                                out=msg[:, nab:DP], in0=sc[:, nab:DP],
                                in1=na_sb[:, tg:tg + 1].broadcast_to(
                                    [128, DP - nab]), op=MULT)
                            if nm == 'c0':
                                for f in range(3):
                                    lo = cfg.HS * g + 3 * f
                                    nc.vector.tensor_tensor(
                                        out=msg[:, lo:lo + 3],
                                        in0=sc[:, lo:lo + 3],
                                        in1=fyo_sb[:, 3 * tg + f:3 * tg + f + 1]
                                        .broadcast_to([128, 3]), op=MULT)
                        S = Sp.tile([128, 128], BF16, tag="S")
                        nc.vector.tensor_scalar(
                            out=S[:, :], in0=iota_bf[:, :],
                            scalar1=dst_sb[:, tg:tg + 1], scalar2=None,
                            op0=ISEQ)
                        for (p0_, p1_) in pieces:
                            nc.tensor.matmul(
                                agps[:, p0_:p1_], S[:, :], msg[:, p0_:p1_],
                                start=(t == 0), stop=(t == nt - 1))
                    # evacuate aggr (DVE)
                    ags = agsp.tile([128, DPMAX], BF16, tag="ags")
                    nc.vector.tensor_copy(ags[:, 0:DP], agps[:, 0:DP])
                    # transposes (groups of 4 into one psum tile), evac on DVE
                    agT = agtp.tile([128, NFULLMAX * 128], BF16, tag="agT")
                    agTt = agtp.tile([TAILMAX, 128], BF16, tag="agTt")
                    for gi in range(0, nfull, 4):
                        gn = min(4, nfull - gi)
                        tp = ps_tp.tile([128, 512], BF16, tag="tp")
                        for u in range(gn):
                            ci = gi + u
                            nc.tensor.transpose(
                                tp[:, u * 128:(u + 1) * 128],
                                ags[:, ci * 128:(ci + 1) * 128],
                                ident[:, :])
                        nc.vector.tensor_copy(
                            agT[:, gi * 128:(gi + gn) * 128],
                            tp[:, 0:gn * 128])
                    if tailk:
                        tp2 = ps_tp.tile([128, 512], BF16, tag="tp")
                        nc.tensor.transpose(
                            tp2[0:tailk, 0:128],
                            ags[:, nfull * 128:nfull * 128 + tailk],
                            ident[:, :])
                        nc.vector.tensor_copy(agTt[0:tailk, :],
                                              tp2[0:tailk, 0:128])
                    # out matmul
                    om = ps_om.tile([128, OCPMAX], F32, tag="om")
                    for ci in range(nfull):
                        nc.tensor.matmul(
                            om[:, 0:OCP],
                            agT[:, ci * 128:(ci + 1) * 128],
                            wt[:, ci * OCP:(ci + 1) * OCP],
                            start=(ci == 0), stop=False)
                    if tailk:
                        nc.tensor.matmul(om[:, 0:OCP], agTt[0:tailk, :],
                                         wtail[0:tailk, 0:OCP],
                                         start=(nfull == 0), stop=False)
                    nc.tensor.matmul(om[:, 0:OCP], ones[:, :],
                                     wbias[:, 0:OCP], start=False, stop=True)
                    if lay['relu']:
                        outs = outp.tile([128, OCPMAX], BF16, tag="outs")
                        omr = outp.tile([128, OCPMAX], BF16, tag="omr")
                        nc.scalar.activation(omr[:, 0:OCP], om[:, 0:OCP], RELU)
                        nc.scalar.activation(outs[:, 0:OCP], omr[:, 0:OCP],
                                             TANH)
                        ro = wofs_of(cfg, w)
                        nc.sync.dma_start(
                            out=oslices[wchunk][ro:ro + wsz, :],
                            in_=outs[0:wsz, 0:HID])
                    else:
                        outs = outp.tile([128, OCPMAX], F32, tag="outsf")
                        nc.scalar.activation(outs[:, 0:OCP], om[:, 0:OCP],
                                             TANH)
                        nc.sync.dma_start(
                            out=out_fin[w * 128:w * 128 + wsz, :],
                            in_=outs[0:wsz, 0:cfg.OUT])
                    # chunk AllGathers
                    if nm != 'c2' and w == cfg.W0 - 1:
                        nc.gpsimd.collective_compute(
                            "AllGather", mybir.AluOpType.bypass,
                            replica_groups=[list(range(NCORES))],
                            ins=[oslices[0][:, :]],
                            outs=[Xout0[:, :]])
                if nm != 'c2':
                    nc.gpsimd.collective_compute(
                        "AllGather", mybir.AluOpType.bypass,
                        replica_groups=[list(range(NCORES))],
                        ins=[oslices[1][:, :]],
                        outs=[Xout1[:, :]])

            for rep in range(repeat):
                for lay in cfg.LAYERS:
                    emit_layer(lay, rep)
    nc.compile()
    return nc


def wofs_of(cfg, w):
    a, b = (0, cfg.W0) if w < cfg.W0 else (cfg.W0, cfg.NWIN)
    return sum(cfg.WSIZES[a:w])


def _run(inputs, trace=False, repeat=1):
    cfg = Cfg()
    struct, per_core, wts, xc0, asm = _preprocess(cfg, inputs)
    nc = _build(cfg, struct, repeat=repeat)
    in_maps = []
    for c in range(NCORES):
        im = dict(per_core[c])
        im['xc0_in'] = xc0
        for k, v in wts.items():
            im[k] = v
        in_maps.append(im)
    res = run_bass_kernel_spmd(nc, in_maps, list(range(NCORES)), trace=trace)
    out = np.zeros((cfg.N, cfg.OUT), np.float32)
    for c in range(NCORES):
        sl = res.results[c]['out_final']
        sel = asm['node_core'] == c
        out[sel] = sl[asm['node_row'][sel]]
    return out, res


def kernel(**inputs):
    return _run(inputs, trace=False)[0]
